# revision 28
# baseline (speedup 1.0000x reference)
"""Paged KV-cache decode attention with ALiBi (Baichuan-style), fused
QKV + attention + output projection, tensor-parallel over heads across
8 Trainium2 NeuronCores.

bf16 design (memory regime: ~41 MB/core HBM traffic, ~358 GB/s/NC cap
=> ~114 us DMA floor):
  - All large tensors cast to bf16 on host, staged in on-chip layouts so
    every DMA is 0.65-4.3 MB with multi-KB contiguous runs per partition.
  - DMA spread over the 3 available rings (SWDGE via gpsimd, HWDGE via
    sync + scalar), emitted in global need-order: per-head q weights +
    K/V(b=0) first, then k/v weights woven with K/V(b=1..3), o_proj
    weights strictly last so its matmuls chase the final stream.
  - qkv computed per-(w,head): weight-stationary bf16 (FWL), one psum
    accumulator per head, directly in transposed layout qT/kT/vT
    [128(d), 5(h)*4(b)].
  - New-token K/V handled WITHOUT cache scatters: host bakes
    bias[pos] = -1e30 (stale cache column contributes 0) and the new
    token's term a_pos = exp(q . k_new) is computed for all 20 (b,h) at
    once (elementwise mul + ones matmul); its rank-1 a_pos * v_new
    contribution is added before normalization.
  - softmax without max-subtraction (scores O(5), exp safe in fp32),
    masking baked into the host-precomputed fp32 additive bias.
  - attention per b software-pipelined over heads (scores h+1 issued
    before attn@V of h) so the exp chain never stalls the PE.
  - o_proj in natural orientation (out [4, 5120]); host sums the 8
    partial outputs (the "all-reduce").
"""

import math
import os
import sys
from contextlib import ExitStack

import numpy as np
import ml_dtypes

sys.path.insert(0, "/opt/trn_rl_repo")

BF16 = ml_dtypes.bfloat16

B = 4
E = 5120
H = 40
D = 128
BS = 16
NB = 512
MB = 128
S = MB * BS  # 2048
NCORES = 8
HPC = H // NCORES   # 5 heads per core
EPC = HPC * D       # 640
NCH = S // 128      # 16 chunks of 128 tokens

NEG = -1.0e30


def _alibi_slopes(num_heads):
    cp2 = 2 ** int(math.floor(math.log2(num_heads)))
    base = 2.0 ** (-(2.0 ** (-(math.log2(cp2) - 3))))
    slopes = base ** np.arange(1, cp2 + 1, dtype=np.float64)
    if cp2 != num_heads:
        extra_base = 2.0 ** (-(2.0 ** (-(math.log2(2 * cp2) - 3))))
        n_rem = min(cp2, num_heads - cp2)
        extra = extra_base ** np.arange(1, 1 + 2 * n_rem, 2, dtype=np.float64)
        slopes = np.concatenate([slopes, extra])
    return slopes.astype(np.float32)


_PROGRAM_CACHE = {}
LAST_RESULTS = None  # BassKernelResults of the most recent run (for test.py)


def _build_program(nch):
    """Build the SPMD Bass program. nch (per-sequence chunk counts) is
    baked statically; all other seq-length dependence lives in host data."""
    import concourse.bacc as bacc
    import concourse.bass as bass
    import concourse.tile as tile
    from concourse import mybir

    f32 = mybir.dt.float32
    f16 = mybir.dt.float16
    bf16 = mybir.dt.bfloat16
    nc = bacc.Bacc()

    hT = nc.declare_dram_parameter("hT", [128, 40 * B], bf16, isOutput=False)
    # per-(w, head) weight streams: [3, 5(h), 128(p), 40(kc)*128(c)]
    qkvw = nc.declare_dram_parameter("qkvw", [3, HPC, 128, 40 * D], bf16, isOutput=False)
    ow = nc.declare_dram_parameter("ow", [5, 128, 2 * HPC * 512], bf16, isOutput=False)
    kt = nc.declare_dram_parameter("kt", [B, 128, HPC, S], bf16, isOutput=False)
    vt = nc.declare_dram_parameter("vt", [B, 128, HPC, NCH, D], bf16, isOutput=False)
    bias = nc.declare_dram_parameter("bias", [128, B * HPC * NCH], f16, isOutput=False)
    out = nc.declare_dram_parameter("out", [B, E], bf16, isOutput=True)

    nmax = max(nch)

    # All bulk DMA goes through the two HWDGE rings (sync + scalar): the
    # SWDGE (gpsimd) path costs a ~6 us queue drain in the kernel epilogue.
    # Greedy byte-balancing keeps both rings' FIFOs on the global
    # need-order while splitting bytes ~50/50.
    ring_bytes = {'sync': 0.0, 'scalar': 0.0}

    def pick_ring(mb):
        # all bulk DMA on the sync ring: a single HWDGE ring sustains
        # ~410 GB/s and leaves the scalar engine free for the exp chain
        return 'sync' 

    with tile.TileContext(nc) as tc, ExitStack() as ctx:
        consts = ctx.enter_context(tc.tile_pool(name="consts", bufs=1))
        wpool = ctx.enter_context(tc.tile_pool(name="wpool", bufs=3))
        kvpool = ctx.enter_context(tc.tile_pool(name="kvpool", bufs=1))
        opool = ctx.enter_context(tc.tile_pool(name="opool", bufs=4))
        tmp = ctx.enter_context(tc.tile_pool(name="tmp", bufs=3))
        psum = ctx.enter_context(tc.tile_pool(name="psum", bufs=8, space="PSUM"))

        def eng(name):
            return getattr(nc, name)

        # ---- constants / small inputs ----
        hT_sb = consts.tile([128, 40 * B], bf16)         # (E%128, (Echunk, b))
        nc.sync.dma_start(out=hT_sb[:], in_=hT[:])
        ring_bytes['sync'] += 0.05
        bias_sb = consts.tile([128, B * HPC * NCH], f16)  # (t%128, (b, h, chunk))
        nc.scalar.dma_start(out=bias_sb[:], in_=bias[:])
        ring_bytes['scalar'] += 0.33
        ones_col = consts.tile([128, 1], f32)
        nc.vector.memset(ones_col[:], 1.0)
        ones_row = consts.tile([1, 128], f32)
        nc.vector.memset(ones_row[:], 1.0)

        qT_sb = consts.tile([128, HPC * B], bf16)   # col = h*B + b ; partition = d
        kT_sb = consts.tile([128, HPC * B], bf16)
        vT_sb = consts.tile([128, HPC * B], bf16)
        colsum_sb = consts.tile([128, HPC * B], f32)
        aoT_sb = consts.tile([128, HPC * B], f32)   # unnormalized attn@V ^T
        out_sb = consts.tile([B, E], bf16)

        # ---- per-(w, head) fused QKV projection (weight-stationary) ----
        def qkv_head(w, h):
            dst = (qT_sb, kT_sb, vT_sb)[w]
            wt = wpool.tile([128, 40 * D], bf16, tag="w")
            eng(pick_ring(1.31)).dma_start(out=wt[:], in_=qkvw[w, h])
            ps = psum.tile([128, B], f32, tag="ps", name=f"ps_qkv{w}_{h}")
            for kc in range(40):
                nc.tensor.matmul(
                    ps[:],
                    lhsT=wt[:, kc * D:(kc + 1) * D],
                    rhs=hT_sb[:, kc * B:(kc + 1) * B],
                    start=(kc == 0),
                    stop=(kc == 39),
                )
            nc.vector.tensor_copy(dst[:, h * B:(h + 1) * B], ps[:])

        # ---- attention for one sequence b (software-pipelined over heads) --
        def attention(b):
            n = nch[b]
            sd = n * 128
            mb = HPC * sd * 128 * 2 / 1e6
            Kt = kvpool.tile([128, HPC, sd], bf16, tag=f"K{b}")
            eng(pick_ring(mb)).dma_start(out=Kt[:], in_=kt[b, :, :, :sd])
            Vt = kvpool.tile([128, HPC, n, D], bf16, tag=f"V{b}")
            eng(pick_ring(mb)).dma_start(out=Vt[:], in_=vt[b, :, :, :n, :])

            def scores(h):
                col = h * B + b
                sc_ps = psum.tile([128, NCH], f32, tag="ps", name=f"sc_{b}_{h}")
                for c in range(n):
                    nc.tensor.matmul(
                        sc_ps[:, c:c + 1],
                        lhsT=Kt[:, h, c * 128:(c + 1) * 128],
                        rhs=qT_sb[:, col:col + 1],
                        start=True,
                        stop=True,
                    )
                s_sb = tmp.tile([128, NCH], f32, tag="s")
                nc.vector.tensor_add(
                    s_sb[:, :n],
                    sc_ps[:, :n],
                    bias_sb[:, (b * HPC + h) * NCH:(b * HPC + h) * NCH + n],
                )
                attn_sb = tmp.tile([128, NCH], bf16, tag="attn", name=f"at_{b}_{h}")
                nc.scalar.activation(
                    attn_sb[:, :n],
                    s_sb[:, :n],
                    func=mybir.ActivationFunctionType.Exp,
                    accum_out=colsum_sb[:, col:col + 1],
                )
                return attn_sb

            def attn_v(h, attn_sb):
                col = h * B + b
                ao_ps = psum.tile([128, 1], f32, tag="ps", name=f"ao_{b}_{h}")
                for c in range(n):
                    nc.tensor.matmul(
                        ao_ps[:],
                        lhsT=Vt[:, h, c, :],
                        rhs=attn_sb[:, c:c + 1],
                        start=(c == 0),
                        stop=(c == n - 1),
                    )
                nc.vector.tensor_copy(aoT_sb[:, col:col + 1], ao_ps[:])

            prev = None
            for h in range(HPC):
                a = scores(h)
                if prev is not None:
                    attn_v(prev[0], prev[1])
                prev = (h, a)
            attn_v(prev[0], prev[1])

        # ---- global need-order: q heads + KV(b0) first, then k heads,
        # then KV(b1..3), then v heads, o_proj weights strictly last so
        # its matmuls chase the final DMA stream ----
        for h in range(HPC):
            qkv_head(0, h)
        attention(0)
        for h in range(HPC):
            qkv_head(1, h)

        # ---- new-token score term, batched over all 20 (b,h): needs only
        # q/k. a_pos = exp(q . k_new) (alibi bias at own position is 0);
        # the stale cache column at pos was killed via bias[pos] = -1e30.
        qk_sb = tmp.tile([128, HPC * B], f32, tag="qk")
        nc.vector.tensor_mul(qk_sb[:], qT_sb[:], kT_sb[:])
        apos_ps = psum.tile([1, HPC * B], f32, tag="ps", name="apos_ps")
        nc.tensor.matmul(apos_ps[:], lhsT=ones_col[:], rhs=qk_sb[:],
                         start=True, stop=True)
        apos_sb = tmp.tile([1, HPC * B], f32, tag="apos")
        nc.scalar.activation(apos_sb[:], apos_ps[:],
                             func=mybir.ActivationFunctionType.Exp)

        attention(1)
        attention(2)
        attention(3)

        # ---- softmax normalization (batched over all 20 (b,h)) ----
        sums_ps = psum.tile([1, HPC * B], f32, tag="ps", name="sums_ps")
        nc.tensor.matmul(sums_ps[:], lhsT=ones_col[:], rhs=colsum_sb[:],
                         start=True, stop=True)
        sums_sb = tmp.tile([1, HPC * B], f32, tag="sums")
        nc.vector.tensor_add(sums_sb[:], sums_ps[:], apos_sb[:])
        recip_sb = tmp.tile([1, HPC * B], f32, tag="recip")
        nc.vector.reciprocal(recip_sb[:], sums_sb[:])
        rb_ps = psum.tile([128, HPC * B], f32, tag="ps", name="rb_ps")
        nc.tensor.matmul(rb_ps[:], lhsT=ones_row[:], rhs=recip_sb[:],
                         start=True, stop=True)
        ap_ps = psum.tile([128, HPC * B], f32, tag="ps", name="ap_ps")
        nc.tensor.matmul(ap_ps[:], lhsT=ones_row[:], rhs=apos_sb[:],
                         start=True, stop=True)
        rb_sb = tmp.tile([128, HPC * B], f32, tag="rb")
        nc.vector.tensor_copy(rb_sb[:], rb_ps[:])
        scaled_ao = tmp.tile([128, HPC * B], f32, tag="sao")
        nc.vector.tensor_mul(scaled_ao[:], aoT_sb[:], rb_sb[:])
        apr_sb = tmp.tile([128, HPC * B], f32, tag="apr")
        nc.vector.tensor_mul(apr_sb[:], ap_ps[:], rb_sb[:])

        # ---- stream tail: v weight DMAs, then o_proj weight DMAs ----
        # (emitted before their consuming compute so the sync ring's FIFO
        # carries them back-to-back with no compute-gated stalls)
        vtiles = []
        for h in range(HPC):
            wt = wpool.tile([128, 40 * D], bf16, tag="w", name=f"wv_{h}")
            eng(pick_ring(1.31)).dma_start(out=wt[:], in_=qkvw[2, h])
            vtiles.append(wt)
        otiles = []
        for jj in range(5):
            ot = opool.tile([128, 2 * HPC * 512], bf16, tag="ot", name=f"ot_{jj}")
            eng(pick_ring(1.31)).dma_start(out=ot[:], in_=ow[jj])
            otiles.append(ot)

        # ---- v projection (chases the v weight stream) ----
        for h in range(HPC):
            ps = psum.tile([128, B], f32, tag="ps", name=f"ps_v_{h}")
            for kc in range(40):
                nc.tensor.matmul(
                    ps[:],
                    lhsT=vtiles[h][:, kc * D:(kc + 1) * D],
                    rhs=hT_sb[:, kc * B:(kc + 1) * B],
                    start=(kc == 0),
                    stop=(kc == 39),
                )
            nc.vector.tensor_copy(vT_sb[:, h * B:(h + 1) * B], ps[:])
        aon_sb = tmp.tile([128, HPC * B], f32, tag="aon")
        nc.vector.tensor_mul(aon_sb[:], apr_sb[:], vT_sb[:])
        attn_nT = consts.tile([128, HPC * B], bf16)
        nc.vector.tensor_add(attn_nT[:], scaled_ao[:], aon_sb[:])

        # ---- output projection (chases the o_proj weight stream) ----
        for jj in range(5):
            ops = [psum.tile([B, 512], f32, tag="ps", name=f"op_{jj}_{ji}")
                   for ji in range(2)]
            for h in range(HPC):
                # ji inner: both matmuls share the stationary attn_nT[h]
                # load, so they pipeline at the N=512 streaming rate
                for ji in range(2):
                    nc.tensor.matmul(
                        ops[ji][:],
                        lhsT=attn_nT[:, h * B:(h + 1) * B],
                        rhs=otiles[jj][:, ji * HPC * 512 + h * 512:
                                       ji * HPC * 512 + (h + 1) * 512],
                        start=(h == 0),
                        stop=(h == HPC - 1),
                    )
            for ji in range(2):
                nc.vector.tensor_copy(
                    out_sb[:, jj * 1024 + ji * 512:jj * 1024 + (ji + 1) * 512],
                    ops[ji][:])

        nc.sync.dma_start(out=out[:], in_=out_sb[:])

    nc.compile()  # Bacc finalize: splits multi-waits (matmul 1-wait limit)
    return nc


def _prepare_core_inputs(core, hidden, qkv_w, o_w, k_cache, v_cache, bt, sl, pos):
    hs = slice(core * HPC, (core + 1) * HPC)
    es = slice(core * EPC, (core + 1) * EPC)

    # qkvw: [3, 5(h), 128(p), 40(kc)*128(c)]; E index e = kc*128 + p,
    # out col = h*128 + c
    qkvw = np.ascontiguousarray(qkv_w[:, :, es])
    qkvw[0] *= np.float32(D ** -0.5)
    qkvw_t = (
        qkvw.reshape(3, 40, 128, HPC, D)
        .transpose(0, 3, 2, 1, 4)
        .reshape(3, HPC, 128, 40 * D)
        .astype(BF16)
    )

    # ow: [10(j), 128(p), 5(h)*512(c')]; row hd = h*128 + p, col = j*512 + c'
    ow_t = (
        np.ascontiguousarray(o_w[es, :])
        .reshape(HPC, 128, 5, 2, 512)
        .transpose(2, 1, 3, 0, 4)
        .reshape(5, 128, 2 * HPC * 512)
        .astype(BF16)
    )

    kg = k_cache[:, hs]  # [NB, HPC, BS, D]
    vg = v_cache[:, hs]
    kt = np.empty((B, 128, HPC, S), BF16)       # [b, d, h, t]
    vt = np.empty((B, 128, HPC, NCH, D), BF16)  # [b, t%128, h, t//128, d]
    for b in range(B):
        kk = kg[bt[b]].transpose(1, 0, 2, 3).reshape(HPC, S, D)
        kt[b] = kk.transpose(2, 0, 1).astype(BF16)
        vv = vg[bt[b]].transpose(1, 0, 2, 3).reshape(HPC, NCH, 128, D)
        vt[b] = vv.transpose(2, 0, 1, 3).astype(BF16)

    slopes = _alibi_slopes(H)[core * HPC:(core + 1) * HPC]
    t_in = np.arange(128)[:, None]
    tg = (np.arange(NCH)[None, :] * 128 + t_in).astype(np.float32)  # [128, 16]
    bias = np.empty((128, B, HPC, NCH), np.float32)
    for b in range(B):
        for h in range(HPC):
            val = slopes[h] * (tg - np.float32(pos[b]))
            val[tg >= sl[b]] = NEG
            val[tg == pos[b]] = NEG  # stale cache col at pos: new-token term is separate
            bias[:, b, h, :] = val

    hTf = np.ascontiguousarray(
        hidden.T.reshape(40, 128, B).transpose(1, 0, 2).reshape(128, 40 * B)
    ).astype(BF16)

    return dict(
        hT=hTf,
        qkvw=qkvw_t,
        ow=ow_t,
        kt=kt,
        vt=vt,
        bias=np.ascontiguousarray(bias.reshape(128, B * HPC * NCH)).astype(np.float16),
    )


def kernel(**inputs):
    global LAST_RESULTS
    hidden = np.asarray(inputs["hidden_states"], np.float32)
    qkv_w = np.asarray(inputs["qkv_weight"], np.float32)
    o_w = np.asarray(inputs["o_proj_weight"], np.float32)
    k_cache = np.asarray(inputs["k_cache"], np.float32)
    v_cache = np.asarray(inputs["v_cache"], np.float32)
    bt = np.asarray(inputs["block_tables"]).astype(np.int64)
    sl = np.asarray(inputs["sequence_lengths"]).astype(np.int64)

    pos = tuple(int(x) - 1 for x in sl)
    nch = tuple(int(math.ceil(int(x) / 128)) for x in sl)

    in_maps = [
        _prepare_core_inputs(c, hidden, qkv_w, o_w, k_cache, v_cache, bt, sl, pos)
        for c in range(NCORES)
    ]

    if nch not in _PROGRAM_CACHE:
        _PROGRAM_CACHE[nch] = _build_program(nch)
    nc = _PROGRAM_CACHE[nch]

    from concourse.bass_utils import run_bass_kernel_spmd

    res = run_bass_kernel_spmd(
        nc,
        in_maps,
        core_ids=list(range(NCORES)),
        trace=bool(os.environ.get("BASS_TRACE")),
    )
    LAST_RESULTS = res

    out = np.zeros((B, E), np.float64)
    for c in range(NCORES):
        out += np.asarray(res.results[c]["out"]).astype(np.float64)
    return out.astype(np.float32)


# revision 29
# speedup vs baseline: 1.0206x; 1.0206x over previous
"""Paged KV-cache decode attention with ALiBi (Baichuan-style), fused
QKV + attention + output projection, tensor-parallel over heads across
8 Trainium2 NeuronCores.

bf16 design (memory regime: ~41 MB/core HBM traffic, ~358 GB/s/NC cap
=> ~114 us DMA floor):
  - All large tensors cast to bf16 on host, staged in on-chip layouts so
    every DMA is 0.65-4.3 MB with multi-KB contiguous runs per partition.
  - DMA spread over the 3 available rings (SWDGE via gpsimd, HWDGE via
    sync + scalar), emitted in global need-order: per-head q weights +
    K/V(b=0) first, then k/v weights woven with K/V(b=1..3), o_proj
    weights strictly last so its matmuls chase the final stream.
  - qkv computed per-(w,head): weight-stationary bf16 (FWL), one psum
    accumulator per head, directly in transposed layout qT/kT/vT
    [128(d), 5(h)*4(b)].
  - New-token K/V handled WITHOUT cache scatters: host bakes
    bias[pos] = -1e30 (stale cache column contributes 0) and the new
    token's term a_pos = exp(q . k_new) is computed for all 20 (b,h) at
    once (elementwise mul + ones matmul); its rank-1 a_pos * v_new
    contribution is added before normalization.
  - softmax without max-subtraction (scores O(5), exp safe in fp32),
    masking baked into the host-precomputed fp32 additive bias.
  - attention per b software-pipelined over heads (scores h+1 issued
    before attn@V of h) so the exp chain never stalls the PE.
  - o_proj in natural orientation (out [4, 5120]); host sums the 8
    partial outputs (the "all-reduce").
"""

import math
import os
import sys
from contextlib import ExitStack

import numpy as np
import ml_dtypes

sys.path.insert(0, "/opt/trn_rl_repo")

BF16 = ml_dtypes.bfloat16

B = 4
E = 5120
H = 40
D = 128
BS = 16
NB = 512
MB = 128
S = MB * BS  # 2048
NCORES = 8
HPC = H // NCORES   # 5 heads per core
EPC = HPC * D       # 640
NCH = S // 128      # 16 chunks of 128 tokens

NEG = -1.0e30


def _alibi_slopes(num_heads):
    cp2 = 2 ** int(math.floor(math.log2(num_heads)))
    base = 2.0 ** (-(2.0 ** (-(math.log2(cp2) - 3))))
    slopes = base ** np.arange(1, cp2 + 1, dtype=np.float64)
    if cp2 != num_heads:
        extra_base = 2.0 ** (-(2.0 ** (-(math.log2(2 * cp2) - 3))))
        n_rem = min(cp2, num_heads - cp2)
        extra = extra_base ** np.arange(1, 1 + 2 * n_rem, 2, dtype=np.float64)
        slopes = np.concatenate([slopes, extra])
    return slopes.astype(np.float32)


_PROGRAM_CACHE = {}
LAST_RESULTS = None  # BassKernelResults of the most recent run (for test.py)


def _build_program(nch):
    """Build the SPMD Bass program. nch (per-sequence chunk counts) is
    baked statically; all other seq-length dependence lives in host data."""
    import concourse.bacc as bacc
    import concourse.bass as bass
    import concourse.tile as tile
    from concourse import mybir

    f32 = mybir.dt.float32
    f16 = mybir.dt.float16
    bf16 = mybir.dt.bfloat16
    nc = bacc.Bacc()

    hT = nc.declare_dram_parameter("hT", [128, 40 * B], bf16, isOutput=False)
    # per-(w, head) weight streams: [3, 5(h), 128(p), 40(kc)*128(c)]
    qkvw = nc.declare_dram_parameter("qkvw", [3, HPC, 128, 40 * D], bf16, isOutput=False)
    ow = nc.declare_dram_parameter("ow", [5, 128, 2 * HPC * 512], bf16, isOutput=False)
    # K/V cache packed host-side per b to exactly the needed tokens so each
    # is ONE full-tensor DMA with 11-16 KB contiguous per-partition runs
    kts = [nc.declare_dram_parameter(f"kt{b}", [128, HPC * nch[b] * 128], bf16,
                                     isOutput=False) for b in range(B)]
    vts = [nc.declare_dram_parameter(f"vt{b}", [128, HPC, nch[b], D], bf16,
                                     isOutput=False) for b in range(B)]
    bias = nc.declare_dram_parameter("bias", [128, B * HPC * NCH], f16, isOutput=False)
    out = nc.declare_dram_parameter("out", [B, E], bf16, isOutput=True)

    nmax = max(nch)

    # All bulk DMA goes through the two HWDGE rings (sync + scalar): the
    # SWDGE (gpsimd) path costs a ~6 us queue drain in the kernel epilogue.
    # Greedy byte-balancing keeps both rings' FIFOs on the global
    # need-order while splitting bytes ~50/50.
    ring_bytes = {'sync': 0.0, 'scalar': 0.0}

    def pick_ring(mb):
        # all bulk DMA on the sync ring: a single HWDGE ring sustains
        # ~410 GB/s and leaves the scalar engine free for the exp chain
        return 'sync' 

    with tile.TileContext(nc) as tc, ExitStack() as ctx:
        consts = ctx.enter_context(tc.tile_pool(name="consts", bufs=1))
        wpool = ctx.enter_context(tc.tile_pool(name="wpool", bufs=4))
        kvpool = ctx.enter_context(tc.tile_pool(name="kvpool", bufs=1))
        opool = ctx.enter_context(tc.tile_pool(name="opool", bufs=3))
        tmp = ctx.enter_context(tc.tile_pool(name="tmp", bufs=3))
        psum = ctx.enter_context(tc.tile_pool(name="psum", bufs=8, space="PSUM"))

        def eng(name):
            return getattr(nc, name)

        # ---- constants / small inputs ----
        hT_sb = consts.tile([128, 40 * B], bf16)         # (E%128, (Echunk, b))
        nc.sync.dma_start(out=hT_sb[:], in_=hT[:])
        ring_bytes['sync'] += 0.05
        bias_sb = consts.tile([128, B * HPC * NCH], f16)  # (t%128, (b, h, chunk))
        nc.scalar.dma_start(out=bias_sb[:], in_=bias[:])
        ring_bytes['scalar'] += 0.33
        ones_col = consts.tile([128, 1], f32)
        nc.vector.memset(ones_col[:], 1.0)
        ones_row = consts.tile([1, 128], f32)
        nc.vector.memset(ones_row[:], 1.0)

        qT_sb = consts.tile([128, HPC * B], bf16)   # col = h*B + b ; partition = d
        kT_sb = consts.tile([128, HPC * B], bf16)
        vT_sb = consts.tile([128, HPC * B], bf16)
        colsum_sb = consts.tile([128, HPC * B], f32)
        aoT_sb = consts.tile([128, HPC * B], f32)   # unnormalized attn@V ^T
        out_sb = consts.tile([B, E], bf16)

        # ---- per-(w, head) fused QKV projection (weight-stationary) ----
        def qkv_head(w, h):
            dst = (qT_sb, kT_sb, vT_sb)[w]
            wt = wpool.tile([128, 40 * D], bf16, tag="w")
            eng(pick_ring(1.31)).dma_start(out=wt[:], in_=qkvw[w, h])
            ps = psum.tile([128, B], f32, tag="ps", name=f"ps_qkv{w}_{h}")
            for kc in range(40):
                nc.tensor.matmul(
                    ps[:],
                    lhsT=wt[:, kc * D:(kc + 1) * D],
                    rhs=hT_sb[:, kc * B:(kc + 1) * B],
                    start=(kc == 0),
                    stop=(kc == 39),
                )
            nc.vector.tensor_copy(dst[:, h * B:(h + 1) * B], ps[:])

        # ---- attention for one sequence b (software-pipelined over heads) --
        def attention(b):
            n = nch[b]
            sd = n * 128
            mb = HPC * sd * 128 * 2 / 1e6
            Kt = kvpool.tile([128, HPC * sd], bf16, tag=f"K{b}")
            eng(pick_ring(mb)).dma_start(out=Kt[:], in_=kts[b][:])
            Vt = kvpool.tile([128, HPC, n, D], bf16, tag=f"V{b}")
            eng(pick_ring(mb)).dma_start(out=Vt[:], in_=vts[b][:])

            def scores(h):
                col = h * B + b
                sc_ps = psum.tile([128, NCH], f32, tag="ps", name=f"sc_{b}_{h}")
                for c in range(n):
                    nc.tensor.matmul(
                        sc_ps[:, c:c + 1],
                        lhsT=Kt[:, h * sd + c * 128:h * sd + (c + 1) * 128],
                        rhs=qT_sb[:, col:col + 1],
                        start=True,
                        stop=True,
                    )
                s_sb = tmp.tile([128, NCH], f32, tag="s")
                nc.vector.tensor_add(
                    s_sb[:, :n],
                    sc_ps[:, :n],
                    bias_sb[:, (b * HPC + h) * NCH:(b * HPC + h) * NCH + n],
                )
                attn_sb = tmp.tile([128, NCH], bf16, tag="attn", name=f"at_{b}_{h}")
                nc.scalar.activation(
                    attn_sb[:, :n],
                    s_sb[:, :n],
                    func=mybir.ActivationFunctionType.Exp,
                    accum_out=colsum_sb[:, col:col + 1],
                )
                return attn_sb

            def attn_v(h, attn_sb):
                col = h * B + b
                ao_ps = psum.tile([128, 1], f32, tag="ps", name=f"ao_{b}_{h}")
                for c in range(n):
                    nc.tensor.matmul(
                        ao_ps[:],
                        lhsT=Vt[:, h, c, :],
                        rhs=attn_sb[:, c:c + 1],
                        start=(c == 0),
                        stop=(c == n - 1),
                    )
                nc.vector.tensor_copy(aoT_sb[:, col:col + 1], ao_ps[:])

            prev = None
            for h in range(HPC):
                a = scores(h)
                if prev is not None:
                    attn_v(prev[0], prev[1])
                prev = (h, a)
            attn_v(prev[0], prev[1])

        # ---- global need-order: q heads + KV(b0) first, then k heads,
        # then KV(b1..3), then v heads, o_proj weights strictly last so
        # its matmuls chase the final DMA stream ----
        for h in range(HPC):
            qkv_head(0, h)
        attention(0)
        for h in range(HPC):
            qkv_head(1, h)

        # ---- new-token score term, batched over all 20 (b,h): needs only
        # q/k. a_pos = exp(q . k_new) (alibi bias at own position is 0);
        # the stale cache column at pos was killed via bias[pos] = -1e30.
        qk_sb = tmp.tile([128, HPC * B], f32, tag="qk")
        nc.vector.tensor_mul(qk_sb[:], qT_sb[:], kT_sb[:])
        apos_ps = psum.tile([1, HPC * B], f32, tag="ps", name="apos_ps")
        nc.tensor.matmul(apos_ps[:], lhsT=ones_col[:], rhs=qk_sb[:],
                         start=True, stop=True)
        apos_sb = tmp.tile([1, HPC * B], f32, tag="apos")
        nc.scalar.activation(apos_sb[:], apos_ps[:],
                             func=mybir.ActivationFunctionType.Exp)

        attention(1)
        attention(2)
        attention(3)

        # ---- softmax normalization (batched over all 20 (b,h)) ----
        sums_ps = psum.tile([1, HPC * B], f32, tag="ps", name="sums_ps")
        nc.tensor.matmul(sums_ps[:], lhsT=ones_col[:], rhs=colsum_sb[:],
                         start=True, stop=True)
        sums_sb = tmp.tile([1, HPC * B], f32, tag="sums")
        nc.vector.tensor_add(sums_sb[:], sums_ps[:], apos_sb[:])
        recip_sb = tmp.tile([1, HPC * B], f32, tag="recip")
        nc.vector.reciprocal(recip_sb[:], sums_sb[:])
        rb_ps = psum.tile([128, HPC * B], f32, tag="ps", name="rb_ps")
        nc.tensor.matmul(rb_ps[:], lhsT=ones_row[:], rhs=recip_sb[:],
                         start=True, stop=True)
        ap_ps = psum.tile([128, HPC * B], f32, tag="ps", name="ap_ps")
        nc.tensor.matmul(ap_ps[:], lhsT=ones_row[:], rhs=apos_sb[:],
                         start=True, stop=True)
        rb_sb = tmp.tile([128, HPC * B], f32, tag="rb")
        nc.vector.tensor_copy(rb_sb[:], rb_ps[:])
        scaled_ao = tmp.tile([128, HPC * B], f32, tag="sao")
        nc.vector.tensor_mul(scaled_ao[:], aoT_sb[:], rb_sb[:])
        apr_sb = tmp.tile([128, HPC * B], f32, tag="apr")
        nc.vector.tensor_mul(apr_sb[:], ap_ps[:], rb_sb[:])

        # ---- stream tail: v weight DMAs, then o_proj weight DMAs ----
        # (emitted before their consuming compute so the sync ring's FIFO
        # carries them back-to-back with no compute-gated stalls)
        vtiles = []
        for h in range(HPC):
            wt = wpool.tile([128, 40 * D], bf16, tag="w", name=f"wv_{h}")
            eng(pick_ring(1.31)).dma_start(out=wt[:], in_=qkvw[2, h])
            vtiles.append(wt)
        otiles = []
        for jj in range(5):
            ot = opool.tile([128, 2 * HPC * 512], bf16, tag="ot", name=f"ot_{jj}")
            eng(pick_ring(1.31)).dma_start(out=ot[:], in_=ow[jj])
            otiles.append(ot)

        # ---- v projection (chases the v weight stream) ----
        for h in range(HPC):
            ps = psum.tile([128, B], f32, tag="ps", name=f"ps_v_{h}")
            for kc in range(40):
                nc.tensor.matmul(
                    ps[:],
                    lhsT=vtiles[h][:, kc * D:(kc + 1) * D],
                    rhs=hT_sb[:, kc * B:(kc + 1) * B],
                    start=(kc == 0),
                    stop=(kc == 39),
                )
            nc.vector.tensor_copy(vT_sb[:, h * B:(h + 1) * B], ps[:])
        aon_sb = tmp.tile([128, HPC * B], f32, tag="aon")
        nc.vector.tensor_mul(aon_sb[:], apr_sb[:], vT_sb[:])
        attn_nT = consts.tile([128, HPC * B], bf16)
        nc.vector.tensor_add(attn_nT[:], scaled_ao[:], aon_sb[:])

        # ---- output projection (chases the o_proj weight stream) ----
        for jj in range(5):
            ops = [psum.tile([B, 512], f32, tag="ps", name=f"op_{jj}_{ji}")
                   for ji in range(2)]
            for h in range(HPC):
                # ji inner: both matmuls share the stationary attn_nT[h]
                # load, so they pipeline at the N=512 streaming rate
                for ji in range(2):
                    nc.tensor.matmul(
                        ops[ji][:],
                        lhsT=attn_nT[:, h * B:(h + 1) * B],
                        rhs=otiles[jj][:, ji * HPC * 512 + h * 512:
                                       ji * HPC * 512 + (h + 1) * 512],
                        start=(h == 0),
                        stop=(h == HPC - 1),
                    )
            for ji in range(2):
                nc.vector.tensor_copy(
                    out_sb[:, jj * 1024 + ji * 512:jj * 1024 + (ji + 1) * 512],
                    ops[ji][:])

        nc.sync.dma_start(out=out[:], in_=out_sb[:])

    nc.compile()  # Bacc finalize: splits multi-waits (matmul 1-wait limit)
    return nc


def _prepare_core_inputs(core, hidden, qkv_w, o_w, k_cache, v_cache, bt, sl, pos):
    hs = slice(core * HPC, (core + 1) * HPC)
    es = slice(core * EPC, (core + 1) * EPC)

    # qkvw: [3, 5(h), 128(p), 40(kc)*128(c)]; E index e = kc*128 + p,
    # out col = h*128 + c
    qkvw = np.ascontiguousarray(qkv_w[:, :, es])
    qkvw[0] *= np.float32(D ** -0.5)
    qkvw_t = (
        qkvw.reshape(3, 40, 128, HPC, D)
        .transpose(0, 3, 2, 1, 4)
        .reshape(3, HPC, 128, 40 * D)
        .astype(BF16)
    )

    # ow: [10(j), 128(p), 5(h)*512(c')]; row hd = h*128 + p, col = j*512 + c'
    ow_t = (
        np.ascontiguousarray(o_w[es, :])
        .reshape(HPC, 128, 5, 2, 512)
        .transpose(2, 1, 3, 0, 4)
        .reshape(5, 128, 2 * HPC * 512)
        .astype(BF16)
    )

    kg = k_cache[:, hs]  # [NB, HPC, BS, D]
    vg = v_cache[:, hs]
    nch = [int(math.ceil(int(x) / 128)) for x in sl]
    kvs = {}
    for b in range(B):
        sd = nch[b] * 128
        kk = kg[bt[b]].transpose(1, 0, 2, 3).reshape(HPC, S, D)[:, :sd]
        # [128(d), HPC*sd]
        kvs[f"kt{b}"] = np.ascontiguousarray(
            kk.transpose(2, 0, 1).reshape(128, HPC * sd)).astype(BF16)
        vv = vg[bt[b]].transpose(1, 0, 2, 3).reshape(HPC, NCH, 128, D)[:, :nch[b]]
        # [128(t%128), HPC, n, D]
        kvs[f"vt{b}"] = np.ascontiguousarray(
            vv.transpose(2, 0, 1, 3)).astype(BF16)

    slopes = _alibi_slopes(H)[core * HPC:(core + 1) * HPC]
    t_in = np.arange(128)[:, None]
    tg = (np.arange(NCH)[None, :] * 128 + t_in).astype(np.float32)  # [128, 16]
    bias = np.empty((128, B, HPC, NCH), np.float32)
    for b in range(B):
        for h in range(HPC):
            val = slopes[h] * (tg - np.float32(pos[b]))
            val[tg >= sl[b]] = NEG
            val[tg == pos[b]] = NEG  # stale cache col at pos: new-token term is separate
            bias[:, b, h, :] = val

    hTf = np.ascontiguousarray(
        hidden.T.reshape(40, 128, B).transpose(1, 0, 2).reshape(128, 40 * B)
    ).astype(BF16)

    return dict(
        hT=hTf,
        qkvw=qkvw_t,
        ow=ow_t,
        bias=np.ascontiguousarray(bias.reshape(128, B * HPC * NCH)).astype(np.float16),
        **kvs,
    )


def kernel(**inputs):
    global LAST_RESULTS
    hidden = np.asarray(inputs["hidden_states"], np.float32)
    qkv_w = np.asarray(inputs["qkv_weight"], np.float32)
    o_w = np.asarray(inputs["o_proj_weight"], np.float32)
    k_cache = np.asarray(inputs["k_cache"], np.float32)
    v_cache = np.asarray(inputs["v_cache"], np.float32)
    bt = np.asarray(inputs["block_tables"]).astype(np.int64)
    sl = np.asarray(inputs["sequence_lengths"]).astype(np.int64)

    pos = tuple(int(x) - 1 for x in sl)
    nch = tuple(int(math.ceil(int(x) / 128)) for x in sl)

    in_maps = [
        _prepare_core_inputs(c, hidden, qkv_w, o_w, k_cache, v_cache, bt, sl, pos)
        for c in range(NCORES)
    ]

    if nch not in _PROGRAM_CACHE:
        _PROGRAM_CACHE[nch] = _build_program(nch)
    nc = _PROGRAM_CACHE[nch]

    from concourse.bass_utils import run_bass_kernel_spmd

    res = run_bass_kernel_spmd(
        nc,
        in_maps,
        core_ids=list(range(NCORES)),
        trace=bool(os.environ.get("BASS_TRACE")),
    )
    LAST_RESULTS = res

    out = np.zeros((B, E), np.float64)
    for c in range(NCORES):
        out += np.asarray(res.results[c]["out"]).astype(np.float64)
    return out.astype(np.float32)
